# revision 18
# baseline (speedup 1.0000x reference)
"""Trainium2 Bass kernel for nn_InputRotationWrapper: y = WHT(x) @ W^T + b.

Algebraic fold: WHT (normalized Walsh-Hadamard along feature dim, H symmetric)
commutes into the weight: y = (x H) W^T = x (W H)^T.  The device runs a pure
GEMM  y = x @ Wr^T + b  with Wr = WHT(W) computed once on the host.

On top of the fold, one level of STRASSEN over 2x2x2 blocking of
(o, k, t) cuts the PE matmul count by 1/8 — the kernel is PE-streaming-bound
at fp16 (1 moving column/cycle), so this is a direct 12.5% win that neither
fp8 (accuracy: e4m3 x,W measures 3.8e-2 rel err vs the 2e-2 gate) nor uint8
(TRN2 silicon zeroes integer matmul products; probed via NEFF dtype patch)
can reach.

  C = Wr @ x^T = [[C11 C12],[C21 C22]],  A = Wr halves, B = x^T halves
  M1=(A11+A22)(B11+B22) M2=(A21+A22)B11 M3=A11(B12-B22) M4=A22(B21-B11)
  M5=(A11+A12)B22 M6=(A21-A11)(B11+B12) M7=(A12-A22)(B21+B22)
  C11=M1+M4-M5+M7  C12=M3+M5  C21=M2+M4  C22=M1-M2+M3+M6

All 7 A-combos (W-side) and 7 B-combos (x-side) are precomputed on the host
in f64/f32 and shipped as fp16: the device only runs products and cheap
recombines.  Per core (1024 tokens, data-parallel over 8 cores):

  - 7 x-combos resident in SBUF: [128p, 16c, 512t] fp16 each (14.7 MB)
  - W-combos streamed per (product j, o-block obp): [128p, 16c, 128o] fp16
  - 16 obp iterations x 7 products x 16-chunk PSUM accumulation
    = 1792 matmuls of 512 cols (vs 2048 classical) ~ 387 us PE wall
  - ScalarE evicts each product PSUM->SBUF fp16; VectorE recombines with
    scalar_tensor_tensor (bias fused via the per-partition scalar operand);
    outputs DMA per [128, 512] slice.  All hidden under PE time.

Startup mirrors the fp16 baseline: PE-clock warmup dummies, then a j-major
group over the first G o-blocks processed c-outer so every arriving x-combo
chunk immediately unlocks G matmuls while the DMA subsystem ramps.
"""
import sys

for _p in ("/opt/trn_rl_repo", "/root/.axon_site/_ro/trn_rl_repo"):
    if _p not in sys.path:
        sys.path.insert(0, _p)

import numpy as np

D = 4096          # feature dim (= rotation size)
TOKENS = 8192     # 4 * 2048
N_CORES = 8
T_CORE = TOKENS // N_CORES   # 1024 tokens per core
P = 128           # partitions
HALF = D // 2     # 2048: o/k half size
KH = HALF // P    # 16 contraction chunks per half
OBH = HALF // P   # 16 output blocks per half
TH = T_CORE // 2  # 512 tokens per t-half (= one matmul moving dim)
NPROD = 7
ORDER = (0, 1, 2, 3, 4, 6, 5)  # product emission order (M6 last: 1-stt tail)

_compiled = None


def _matmul_hadU_np(x: np.ndarray) -> np.ndarray:
    """Normalized WHT along the last axis — exact port of the reference
    recursive-butterfly (K == 1 branch), in float64."""
    n = x.shape[-1]
    shape = x.shape
    v = x.reshape(-1, n, 1)
    while v.shape[1] > 1:
        b_, m, c = v.shape
        v = v.reshape(b_, m // 2, 2, c)
        a, b = v[:, :, 0, :], v[:, :, 1, :]
        v = np.concatenate([a + b, a - b], axis=-1)
    return v.reshape(shape) / np.sqrt(n)


def _build_nc():
    import concourse.tile as tile
    from concourse import bacc, mybir

    dt = mybir.dt
    alu = mybir.AluOpType
    nc = bacc.Bacc(None, target_bir_lowering=False)

    xc_d = nc.dram_tensor("xc", [NPROD, P, KH, TH], dt.float16,
                          kind="ExternalInput")
    wc_d = nc.dram_tensor("wc", [NPROD, OBH, P, KH, P], dt.float16,
                          kind="ExternalInput")
    b_d = nc.dram_tensor("bias", [P, 2 * OBH], dt.float32,
                         kind="ExternalInput")
    y_d = nc.dram_tensor("yt", [D, T_CORE], dt.float16, kind="ExternalOutput")

    G = 4     # startup group: o-blocks processed c-outer per product so each
              # arriving x-combo chunk unlocks G matmuls during the DMA ramp
    WRING = 8   # W tile ring (4 KB/partition each)
    MRING = 24  # staged-product ring (1 KB/partition each); incremental
                # recombine frees most of the startup group by j=3

    with tile.TileContext(nc) as tc:
        with (
            tc.tile_pool(name="xcp", bufs=1) as xcp,
            tc.tile_pool(name="wp", bufs=WRING) as wp,
            tc.tile_pool(name="mp", bufs=MRING) as mp,
            tc.tile_pool(name="tp", bufs=12) as tp,
            tc.tile_pool(name="op", bufs=4) as op,
            tc.tile_pool(name="bp", bufs=1) as bp,
            tc.tile_pool(name="pp", bufs=8, space="PSUM") as pp,
        ):
            b_sb = bp.tile([P, 2 * OBH], dt.float32)

            xc_sb = [
                xcp.tile([P, KH, TH], dt.float16, name=f"xc_{j}")
                for j in range(NPROD)
            ]

            # ---- PE clock warmup (HAM ramps over ~3.4us of activity) ----
            dum = bp.tile([P, 256], dt.float16, tag="dum", name="dum")
            nc.vector.memset(dum[:], 0.0)

            w_tiles = {}

            def w_alloc(j, obp):
                t = wp.tile([P, KH, P], dt.float16, tag="w",
                            name=f"w_{j}_{obp}")
                w_tiles[(j, obp)] = t
                return t

            def w_load(j, obp, eng=None):
                t = w_alloc(j, obp)
                (eng or nc.gpsimd).dma_start(t[:], wc_d[j, obp, :, :, :])
                return t

            def xc_load(j, c0, n, eng=None):
                (eng or nc.scalar).dma_start(
                    xc_sb[j][:, c0:c0 + n, :], xc_d[j, :, c0:c0 + n, :])

            # ---- DMA triggers in arrival-need order ----
            # A single HWDGE queue sustains only ~146 GB/s while the startup
            # needs ~300 GB/s (W 145 + x-combos 152), so BOTH streams are
            # spread across all three queues (scalar/sync/gpsimd, ~99 GB/s
            # each) with round-robin assignment.
            QS = (nc.gpsimd, nc.scalar, nc.sync)

            def w_eng(j, obp):
                return QS[(j + obp) % 3]

            def w_load_r(j, obp):
                return w_load(j, obp, eng=w_eng(j, obp))

            def xc_pieces(j, chunks=((0, 5), (5, 5), (10, 6))):
                for i, (c0, n) in enumerate(chunks):
                    xc_load(j, c0, n, eng=QS[(j + i) % 3])

            nc.sync.dma_start(b_sb[:], b_d[:])
            # xc0 + w0 finely chunked for the DMA ramp
            xc_load(0, 0, 1, eng=QS[0])
            xc_load(0, 5, 1, eng=QS[1])
            xc_load(0, 10, 2, eng=QS[2])
            for q in range(2):
                for gob in range(G):
                    t = w_alloc(0, gob) if q == 0 else w_tiles[(0, gob)]
                    w_eng(0, gob).dma_start(
                        t[:, q * 4:(q + 1) * 4, :],
                        wc_d[0, gob, :, q * 4:(q + 1) * 4, :])
            xc_load(0, 1, 2, eng=QS[0])
            xc_load(0, 6, 2, eng=QS[1])
            xc_load(0, 12, 2, eng=QS[2])
            for gob in range(G):
                w_eng(0, gob).dma_start(
                    w_tiles[(0, gob)][:, 8:16, :], wc_d[0, gob, :, 8:16, :])
            xc_load(0, 3, 2, eng=QS[0])
            xc_load(0, 8, 2, eng=QS[1])
            xc_load(0, 14, 2, eng=QS[2])
            xc_pieces(1)
            for gob in range(G):
                w_load_r(1, gob)
            xc_pieces(2)
            for gob in range(G):
                w_load_r(2, gob)
            xc_pieces(3)
            for gob in range(G):
                w_load_r(3, gob)
            xc_pieces(4)
            for gob in range(G):
                w_load_r(4, gob)
            xc_pieces(6)
            for gob in range(G):
                w_load_r(6, gob)
            xc_pieces(5)
            for gob in range(G):
                w_load_r(5, gob)

            # startup W for the first steady block so obp=G starts clean
            for j in ORDER:
                w_load_r(j, G)

            # ---- PE warmup dummies ----
            ps_warm = pp.tile([P, TH], dt.float32, tag="ps", name="ps_w")
            for _ in range(14):
                nc.tensor.matmul(
                    ps_warm[:, 0:256], dum[:, 0:128], dum[:, 0:256],
                    start=True, stop=True,
                )

            stage = {}

            def evict(j, obp, ps):
                m = mp.tile([P, TH], dt.float16, tag="m", name=f"m_{j}_{obp}")
                nc.scalar.copy(m[:], ps[:])
                stage[(j, obp)] = m
                return m

            def product(j, obp, ps=None):
                if ps is None:
                    ps = pp.tile([P, TH], dt.float32, tag="ps",
                                 name=f"ps_{j}_{obp}")
                wt = w_tiles.pop((j, obp))
                for c in range(KH):
                    nc.tensor.matmul(
                        ps[:], wt[:, c, :], xc_sb[j][:, c, :],
                        start=(c == 0), stop=(c == KH - 1),
                    )
                evict(j, obp, ps)

            # Incremental recombine: emit each scalar_tensor_tensor as soon
            # as its staged inputs exist (called with the just-finished j),
            # all on the vector ALU (gpsimd/Pool lacks TensorScalarPtr on
            # NC-v3).  Products run in ORDER = [0,1,2,3,4,6,5] and C22 is
            # built as (M1-M2+bb) + M3 earlier, so after the LAST product of
            # every o-block only evict -> one stt -> DMA remains.
            #   C11 = M1+M4-M5+M7+bt   C12 = M3+M5+bt
            #   C21 = M2+M4+bb         C22 = ((M1-M2+bb) + M3) + M6
            rec = {}

            def recombine_step(obp, j):
                bt = b_sb[:, obp:obp + 1]
                bb = b_sb[:, OBH + obp:OBH + obp + 1]
                m = lambda k: stage[(k, obp)]
                rt = slice(obp * P, (obp + 1) * P)
                rb = slice((OBH + obp) * P, (OBH + obp + 1) * P)
                r = rec.setdefault(obp, {})

                def tl(pool, tag, nm):
                    return pool.tile([P, TH], dt.float16, tag=tag,
                                     name=f"{nm}_{obp}")

                if j == 1:
                    r["t3"] = tl(tp, "t", "t3")
                    nc.vector.scalar_tensor_tensor(
                        r["t3"][:], m(0)[:], bb, m(1)[:], alu.add, alu.subtract)
                elif j == 2:
                    r["t5"] = tl(tp, "t", "t5")
                    nc.vector.scalar_tensor_tensor(
                        r["t5"][:], r["t3"][:], 0.0, m(2)[:], alu.add, alu.add)
                elif j == 3:
                    r["t1"] = tl(tp, "t", "t1")
                    nc.vector.scalar_tensor_tensor(
                        r["t1"][:], m(0)[:], bt, m(3)[:], alu.add, alu.add)
                    o21 = tl(op, "o", "o21")
                    nc.vector.scalar_tensor_tensor(
                        o21[:], m(1)[:], bb, m(3)[:], alu.add, alu.add)
                    nc.sync.dma_start(y_d[rb, 0:TH], o21[:])
                elif j == 4:
                    o12 = tl(op, "o", "o12")
                    nc.vector.scalar_tensor_tensor(
                        o12[:], m(2)[:], bt, m(4)[:], alu.add, alu.add)
                    nc.sync.dma_start(y_d[rt, TH:T_CORE], o12[:])
                elif j == 6:
                    t2 = tl(tp, "t", "t2")
                    nc.vector.scalar_tensor_tensor(
                        t2[:], m(6)[:], 0.0, m(4)[:], alu.add, alu.subtract)
                    o11 = tl(op, "o", "o11")
                    nc.vector.scalar_tensor_tensor(
                        o11[:], r["t1"][:], 0.0, t2[:], alu.add, alu.add)
                    nc.sync.dma_start(y_d[rt, 0:TH], o11[:])
                elif j == 5:
                    o22 = tl(op, "o", "o22")
                    nc.vector.scalar_tensor_tensor(
                        o22[:], r["t5"][:], 0.0, m(5)[:], alu.add, alu.add)
                    nc.sync.dma_start(y_d[rb, TH:T_CORE], o22[:])
                    for k in range(NPROD):
                        del stage[(k, obp)]
                    del rec[obp]

            # ---- startup group: j-major, c-outer across obp 0..G-1 ----
            for j in ORDER:
                ps_j = []
                for gob in range(G):
                    if j == 0 and gob == 0:
                        ps_j.append(ps_warm)
                    else:
                        ps_j.append(pp.tile(
                            [P, TH], dt.float32, tag="ps",
                            name=f"ps_{j}_{gob}"))
                for c in range(KH):
                    for gob in range(G):
                        nc.tensor.matmul(
                            ps_j[gob][:],
                            w_tiles[(j, gob)][:, c, :], xc_sb[j][:, c, :],
                            start=(c == 0), stop=(c == KH - 1),
                        )
                for gob in range(G):
                    evict(j, gob, ps_j[gob])
                for gob in range(G):
                    recombine_step(gob, j)
            for j, gob in list(w_tiles):
                if gob < G:
                    del w_tiles[(j, gob)]

            # ---- steady state: obp-major ----
            for obp in range(G, OBH):
                for j in ORDER:
                    if obp + 1 < OBH:
                        w_load_r(j, obp + 1)
                    product(j, obp)
                    recombine_step(obp, j)

    nc.compile()
    return nc


def _get_nc():
    global _compiled
    if _compiled is None:
        _compiled = _build_nc()
    return _compiled


def _prep_inputs(x, W, b):
    x = np.asarray(x, dtype=np.float32)
    W = np.asarray(W, dtype=np.float32)
    b = np.asarray(b, dtype=np.float32)

    Wr = _matmul_hadU_np(W.astype(np.float64))  # [o, k] float64
    A11 = Wr[:HALF, :HALF]
    A12 = Wr[:HALF, HALF:]
    A21 = Wr[HALF:, :HALF]
    A22 = Wr[HALF:, HALF:]
    WCs = (A11 + A22, A21 + A22, A11, A22, A11 + A12, A21 - A11, A12 - A22)
    # pack[j][obp, p, c, jo] = WC_j[obp*128 + jo, c*128 + p]
    wc = np.stack([
        w.reshape(OBH, P, KH, P).transpose(0, 3, 2, 1) for w in WCs
    ]).astype(np.float16)
    wc = np.ascontiguousarray(wc)

    b_pack = np.ascontiguousarray(b.reshape(2 * OBH, P).T)  # [128, 32]

    xt = x.reshape(N_CORES, T_CORE, D).transpose(0, 2, 1)  # [core, k, t] f32
    B11 = xt[:, :HALF, :TH]
    B12 = xt[:, :HALF, TH:]
    B21 = xt[:, HALF:, :TH]
    B22 = xt[:, HALF:, TH:]
    XCs = (B11 + B22, B11, B12 - B22, B21 - B11, B22, B11 + B12, B21 + B22)
    # pack[core, j, p, c, t] = XC_j[core, c*128 + p, t]
    xc = np.stack([
        c.reshape(N_CORES, KH, P, TH).transpose(0, 2, 1, 3) for c in XCs
    ], axis=1).astype(np.float16)
    xc = np.ascontiguousarray(xc)

    in_maps = [
        {"xc": xc[i], "wc": wc, "bias": b_pack} for i in range(N_CORES)
    ]
    return in_maps


def _assemble(results):
    # yt per core: [4096 o, 1024 t] fp16 -> y[t, o] fp32
    parts = [r["yt"].T.astype(np.float32) for r in results]
    y = np.concatenate(parts, axis=0)  # [8192, 4096]
    return y.reshape(4, 2048, D)


def _run(x, W, b, **spmd_kwargs):
    from concourse.bass_utils import run_bass_kernel_spmd

    nc = _get_nc()
    in_maps = _prep_inputs(x, W, b)
    res = run_bass_kernel_spmd(nc, in_maps, list(range(N_CORES)), **spmd_kwargs)
    return _assemble(res.results), res


def kernel(x, W, b):
    out, _ = _run(x, W, b)
    return out
